# revision 20
# baseline (speedup 1.0000x reference)
"""Single-head attention (B=4, N=2048, D=1024), scores scaled by 10.

Sharding: 8 cores = (batch, query-half). Core 2b+h owns queries
[1024h:1024(h+1)] of batch b. Pure SPMD — no collectives.

Algebra: scores = Q K^T = x_q (Wq^T Wk) x_k^T, so G = q_w^T @ k_w is
precomputed on host and the kernel computes U = x_q G on device; the
key side of QK^T is raw x. The V path is re-associated:
out = softmax(S) (x Wv) == (softmax(S) x) Wv, so each core computes
Y = P x_full then O = Y Wv locally — no V projection, no AllGathers,
no cross-core dependencies. Flop count is identical.

Key order is permuted (own query half first) so x_q is a column slice
of the x^T tile — softmax is permutation-invariant and Y uses the
same row permutation of x, so the output is unchanged.

Inputs are packed on host so every DMA is 128 partition-contiguous
multi-KB runs (desc-dominated 1KB loads made the old startup 14.5us),
split across both HWDGE rings (sync: x^T + x, scalar: G + Wv) so the
first matmul's operands land before the ~7.4us engine preamble ends.

Numerics: fp16 operands, fp32 PSUM (the x10 score scale needs fp16's
10-bit mantissa; measured rel err ~4.6e-3 vs the 2e-2 gate). The
1/sum normalization happens on the HOST: the kernel emits
unnormalized O^T plus per-query sum rows.

Schedule: QK runs in four 256-query chunks (N=256 matmuls measure
~111ns, at parity with N=512 per-flop; the fp32 score tiles then fit
SBUF with bufs=2). Scores stay k-partitioned so Y consumes P with no
transposes. Y processes chunk PAIRS: the two 256-halves accumulate
into one [128,512] PSUM bank with kt-interleaved matmuls (shares each
LDWEIGHTS across both halves and halves the PSUM->SBUF casts); O runs
N=512 per pair. Tensor stream: U0 U1 QK0 QK1 QK2 QK3 Y01 O01 Y23 O23
with each chunk's softmax (fold-max, rank-1 max broadcast, scale+exp)
interleaved into the following tensor block. Per-query sums ride the
otherwise-idle gpsimd engine. Queue discipline: input loads + fold
DMAs on the sync ring, G/Wv loads + output stores on the scalar ring,
sum rows on gpsimd.
"""

import numpy as np

B, SEQ, D = 4, 2048, 1024
NQ = 1024          # queries per core
QCH = 256          # attention q-chunk (QK granularity)
NCH = NQ // QCH    # 4
NCORES = 8
DT = D // 128      # 8 d-tiles
KT = SEQ // 128    # 16 k-tiles

_BUILT = {}


def _build():
    if "nc" in _BUILT:
        return _BUILT["nc"]
    from contextlib import ExitStack

    import concourse.bass as bass  # noqa: F401
    import concourse.mybir as mybir
    import concourse.tile as tile
    from concourse import bacc, bass_isa

    dt = mybir.dt
    F32, F16 = dt.float32, dt.float16
    AL = mybir.AluOpType
    EXP = mybir.ActivationFunctionType.Exp

    nc = bacc.Bacc("TRN2", target_bir_lowering=False, debug=False)

    # packed DRAM inputs: [128 partitions, contiguous per-partition payload]
    xk_d = nc.dram_tensor("xk", [128, 32 * 512], F16, kind="ExternalInput")
    xv_d = nc.dram_tensor("xv", [128, 128 * 128], F16, kind="ExternalInput")
    g_d = nc.dram_tensor("g", [128, 64 * 128], F16, kind="ExternalInput")
    wv_d = nc.dram_tensor("wv", [128, 64 * 128], F16, kind="ExternalInput")
    ot_d = nc.dram_tensor("ot", [D, NQ], F16, kind="ExternalOutput")
    sm_d = nc.dram_tensor("sm", [NCH, QCH], F32, kind="ExternalOutput")

    xk_a = xk_d.ap().rearrange("p (t n) -> p t n", n=512)    # t = 8*chunk+dti
    xv_a = xv_d.ap().rearrange("p (t e) -> p t e", e=128)    # t = 8*kt+dti
    g_a = g_d.ap().rearrange("p (t e) -> p t e", e=128)      # t = 8*et+dti
    wv_a = wv_d.ap().rearrange("p (t e) -> p t e", e=128)
    ot_r = ot_d.ap().rearrange("(t p) q -> p t q", p=128)

    with tile.TileContext(nc) as tc, ExitStack() as ctx:
        main_pool = ctx.enter_context(tc.tile_pool(name="main", bufs=1))
        xk_t = main_pool.tile([128, 32, 512], F16, tag="xk")
        xv_t = main_pool.tile([128, 128, 128], F16, tag="xv")
        wv_t = main_pool.tile([128, 64, 128], F16, tag="wv")
        uth = main_pool.tile([128, DT, NQ], F16, tag="uth")
        p_t = main_pool.tile([128, KT, NQ], F16, tag="p")

        const_pool = ctx.enter_context(tc.tile_pool(name="const", bufs=1))
        ones_t = const_pool.tile([128, 1], F16, tag="ones")
        nc.vector.memset(ones_t[:], 1.0)

        # ---- Phase A: loads + U projection -------------------------------
        with (
            tc.tile_pool(name="gp", bufs=1) as gpool,
            tc.tile_pool(name="psA", bufs=4, space="PSUM") as psA,
            tc.tile_pool(name="psW", bufs=1, space="PSUM") as psW,
        ):
            g_t = gpool.tile([128, 64, 128], F16, tag="g")
            # ~3.6us of rank-1 warmups fill the initial DMA wait (PE would
            # be idle anyway) and flip the HAM clock gate to 2.4GHz before
            # the first real matmul
            warm = gpool.tile([128, 128], F16, tag="warm")
            nc.vector.memset(warm[:], 0.0)
            warm_ps = psW.tile([128, 128], F32, tag="warmps")
            for _ in range(48):
                nc.tensor.matmul(warm_ps[:], warm[:], warm[:], start=True, stop=True)
            # input loads, first-use order; two HWDGE rings in parallel.
            # The first U group's operands are split fine so its first
            # matmuls can start on half the data
            nc.sync.dma_start(xk_t[:, 0:8, :], xk_a[:, 0:8, :])      # xq chunk 0
            for et in range(DT):
                nc.scalar.dma_start(
                    g_t[:, 8 * et : 8 * et + 8, :], g_a[:, 8 * et : 8 * et + 8, :]
                )
            nc.sync.dma_start(xk_t[:, 8:16, :], xk_a[:, 8:16, :])    # xq chunk 1
            nc.sync.dma_start(xk_t[:, 16:24, :], xk_a[:, 16:24, :])  # other half
            nc.sync.dma_start(xk_t[:, 24:32, :], xk_a[:, 24:32, :])
            for j in range(4):
                nc.sync.dma_start(
                    xv_t[:, 32 * j : 32 * j + 32, :], xv_a[:, 32 * j : 32 * j + 32, :]
                )
            nc.scalar.dma_start(wv_t[:, 0:32, :], wv_a[:, 0:32, :])
            nc.scalar.dma_start(wv_t[:, 32:64, :], wv_a[:, 32:64, :])

            # U^T = (x_q G)^T, two 512-query chunks
            for chn in range(2):
                for et in range(DT):
                    ps = psA.tile([128, 512], F32, tag="psA")
                    for dti in range(DT):
                        nc.tensor.matmul(
                            ps[:],
                            g_t[:, 8 * et + dti, :],
                            xk_t[:, 8 * chn + dti, :],
                            start=(dti == 0),
                            stop=(dti == DT - 1),
                        )
                    nc.vector.tensor_copy(uth[:, et, 512 * chn : 512 * chn + 512], ps[:])

        # ---- Phase B: attention ------------------------------------------
        with (
            tc.tile_pool(name="stp", bufs=3) as stpool,
            tc.tile_pool(name="yp", bufs=1) as ypool,
            tc.tile_pool(name="aux", bufs=2) as auxpool,
            tc.tile_pool(name="osb", bufs=3) as outpool,
            tc.tile_pool(name="psS", bufs=3, space="PSUM") as psS,
            tc.tile_pool(name="psY", bufs=2, space="PSUM") as psY,
            tc.tile_pool(name="psO", bufs=2, space="PSUM") as psO,
            tc.tile_pool(name="psX", bufs=1, space="PSUM") as psX,
        ):
            def qk_group(c, st, kt, rmax):
                # QK^T for one k-tile; running per-query max rides the copy
                q0 = QCH * c
                ps = psS.tile([128, QCH], F32, tag="psS")
                for dti in range(DT):
                    nc.tensor.matmul(
                        ps[:],
                        xk_t[:, 8 * (kt // 4) + dti, 128 * (kt % 4) : 128 * (kt % 4) + 128],
                        uth[:, dti, q0 : q0 + QCH],
                        start=(dti == 0),
                        stop=(dti == DT - 1),
                    )
                nc.scalar.copy(st[:, kt, :], ps[:])
                if kt == 1:
                    nc.vector.tensor_max(rmax[:], st[:, 0, :], st[:, 1, :])
                elif kt >= 2:
                    nc.vector.tensor_max(rmax[:], rmax[:], st[:, kt, :])

            def reduce_bcast(c):
                # per-query max across the 128 key-partitions, broadcast to
                # all partitions, in ONE gpsimd op (replaces a ~6us serial
                # fold tree + rank-1 broadcast matmul on the PE FIFO)
                maxb_ = auxpool.tile([128, QCH], F32, tag="maxb", name=f"maxb{c}")
                nc.gpsimd.partition_all_reduce(
                    maxb_[:], rmax[c][:], 128, bass_isa.ReduceOp.max
                )
                return maxb_

            def stt_exp(c, st, maxb, j):
                # exp(10*s - 10*max) for k-tiles 4j..4j+3 into the shared P
                # buffer
                q0 = QCH * c
                for kt in range(4 * j, 4 * j + 4):
                    nc.vector.scalar_tensor_tensor(
                        st[:, kt, :],
                        st[:, kt, :],
                        1.0,
                        maxb[:],
                        op0=AL.mult,
                        op1=AL.subtract,
                    )
                nc.scalar.activation(
                    p_t[:, 4 * j : 4 * j + 4, q0 : q0 + QCH],
                    st[:, 4 * j : 4 * j + 4, :],
                    EXP,
                )

            def y_group(cp, y_t, dti):
                # Y^T d-tile for chunk PAIR cp: both 256-halves accumulate in
                # one [128,512] PSUM bank, kt-interleaved so each xv weight
                # load serves two matmuls; one cast out.
                q0 = 512 * cp
                ps = psY.tile([128, 512], F32, tag="psY")
                for kt in range(KT):
                    for h in range(2):
                        # start=True clears has_written BANK-wide, so only the
                        # very first matmul into the bank may set it (the h=1
                        # kt=0 half then overwrites its still-clear elements)
                        nc.tensor.matmul(
                            ps[:, 256 * h : 256 * h + 256],
                            xv_t[:, 8 * kt + dti, :],
                            p_t[:, kt, q0 + 256 * h : q0 + 256 * h + 256],
                            start=(kt == 0 and h == 0),
                            stop=(kt == KT - 1 and h == 1),
                        )
                nc.vector.tensor_copy(y_t[:, dti, :], ps[:])

            def o_group(cp, y_t, et):
                # O^T e-tile for chunk pair cp at N=512; store via scalar ring
                q0 = 512 * cp
                ps = psO.tile([128, 512], F32, tag="psO")
                for dti in range(DT):
                    nc.tensor.matmul(
                        ps[:],
                        wv_t[:, 8 * et + dti, :],
                        y_t[:, dti, :],
                        start=(dti == 0),
                        stop=(dti == DT - 1),
                    )
                osb = outpool.tile([128, 512], F16, tag="osb")
                nc.vector.tensor_copy(osb[:], ps[:])
                nc.scalar.dma_start(ot_r[:, et, q0 : q0 + 512], osb[:])

            def sum_mms(c):
                # per-query sums as 16 rank-1 matmuls (ones^T @ P) straight
                # into a [1, QCH] PSUM row: keeps gpsimd off the SBUF ports
                # (its tensor_adds halved DVE throughput via port contention)
                q0 = QCH * c
                qs = psX.tile([1, QCH], F32, tag="qsum")
                for kt in range(KT):
                    nc.tensor.matmul(
                        qs[:],
                        ones_t[:],
                        p_t[:, kt, q0 : q0 + QCH],
                        start=(kt == 0),
                        stop=(kt == KT - 1),
                    )
                srow = auxpool.tile([1, QCH], F32, tag="srow")
                nc.vector.tensor_copy(srow[:], qs[:])
                nc.gpsimd.dma_start(sm_d.ap()[c : c + 1, :], srow[:])

            st = [None] * NCH
            y = [None] * 2
            rmax = [None] * NCH
            maxb = [None] * NCH

            def new_chunk(c):
                st[c] = stpool.tile([128, KT, QCH], F32, tag="st", name=f"st{c}")
                rmax[c] = auxpool.tile([128, QCH], F32, tag="rmax", name=f"rmax{c}")

            def qk_run(c, kts):
                for kt in kts:
                    qk_group(c, st[c], kt, rmax[c])

            def exp_batch(c, j):
                stt_exp(c, st[c], maxb[c], j)

            def y_run(cp, dtis):
                if y[cp] is None:
                    y[cp] = ypool.tile([128, DT, 512], F16, tag="y", name=f"y{cp}")
                for dti in dtis:
                    y_group(cp, y[cp], dti)

            # ---- interleaved schedule ----
            # tensor: QK0 QK1 QK2 QK3 Y01 O01 Y23 O23. Each chunk's softmax
            # is spread over the following TWO half-blocks so the vector
            # engine (scale+sub, running-max, fold trees) never outruns its
            # window; st bufs=3 keeps the pool rotation ahead of the spread.
            new_chunk(0)
            qk_run(0, range(KT))
            maxb[0] = reduce_bcast(0)
            new_chunk(1)
            qk_run(1, range(0, 7))
            exp_batch(0, 0)
            qk_run(1, range(7, 11))
            exp_batch(0, 1)
            qk_run(1, range(11, KT))
            maxb[1] = reduce_bcast(1)
            new_chunk(2)
            qk_run(2, range(0, 3))
            exp_batch(0, 2)
            qk_run(2, range(3, 6))
            exp_batch(0, 3)
            qk_run(2, range(6, 9))
            exp_batch(1, 0)
            qk_run(2, range(9, 12))
            exp_batch(1, 1)
            qk_run(2, range(12, KT))
            maxb[2] = reduce_bcast(2)
            new_chunk(3)
            qk_run(3, range(0, 3))
            exp_batch(1, 2)
            qk_run(3, range(3, 5))
            sum_mms(0)
            exp_batch(1, 3)
            qk_run(3, range(5, 8))
            exp_batch(2, 0)
            qk_run(3, range(8, 11))
            exp_batch(2, 1)
            qk_run(3, range(11, KT))
            maxb[3] = reduce_bcast(3)
            y_run(0, range(0, 1))
            exp_batch(2, 2)
            y_run(0, range(1, 2))
            sum_mms(1)
            exp_batch(2, 3)
            y_run(0, range(2, 3))
            exp_batch(3, 0)
            y_run(0, range(3, 5))
            exp_batch(3, 1)
            y_run(0, range(5, DT))
            o_group(0, y[0], 0)
            exp_batch(3, 2)
            o_group(0, y[0], 1)
            sum_mms(2)
            exp_batch(3, 3)
            for et in range(2, DT):
                o_group(0, y[0], et)
            y_run(1, range(0, 1))
            sum_mms(3)
            y_run(1, range(1, DT))
            for et in range(DT - 1):
                o_group(1, y[1], et)
            ps_l = psO.tile([128, 512], F32, tag="psO", name="ps_last")
            for h in range(2):
                for dti in range(DT):
                    nc.tensor.matmul(
                        ps_l[:, 256 * h : 256 * h + 256],
                        wv_t[:, 8 * (DT - 1) + dti, :],
                        y[1][:, dti, 256 * h : 256 * h + 256],
                        start=(dti == 0 and h == 0),
                        stop=(dti == DT - 1 and h == 1),
                    )
                osb_h = outpool.tile([128, 256], F16, tag="osbh", name=f"osbh{h}")
                nc.vector.tensor_copy(osb_h[:], ps_l[:, 256 * h : 256 * h + 256])
                nc.scalar.dma_start(
                    ot_r[:, DT - 1, 512 + 256 * h : 512 + 256 * h + 256], osb_h[:]
                )

    nc.compile()
    _BUILT["nc"] = nc
    return nc


def _prep_inputs(x, q_w, k_w, v_w):
    f16 = np.float16
    G = (10.0 * (q_w.T @ k_w)).astype(f16)
    g_pack = np.ascontiguousarray(
        G.reshape(8, 128, 8, 128).transpose(1, 2, 0, 3).reshape(128, 64 * 128)
    )
    wv = v_w.T.astype(f16)
    wv_pack = np.ascontiguousarray(
        wv.reshape(8, 128, 8, 128).transpose(1, 2, 0, 3).reshape(128, 64 * 128)
    )

    in_maps = []
    for core in range(NCORES):
        b, h = divmod(core, 2)
        xb = np.asarray(x[b]).astype(f16)                    # [2048, 1024]
        xp = np.concatenate([xb[NQ * h : NQ * (h + 1)], xb[NQ * (1 - h) : NQ * (2 - h)]])
        xk_pack = np.ascontiguousarray(
            xp.T.reshape(8, 128, 4, 512).transpose(1, 2, 0, 3).reshape(128, 32 * 512)
        )
        xv_pack = np.ascontiguousarray(
            xp.reshape(16, 128, 8, 128).transpose(1, 0, 2, 3).reshape(128, 128 * 128)
        )
        in_maps.append({"xk": xk_pack, "xv": xv_pack, "g": g_pack, "wv": wv_pack})
    return in_maps


def run(x, q_w, k_w, v_w, trace=False):
    from concourse.bass_utils import run_bass_kernel_spmd

    nc = _build()
    in_maps = _prep_inputs(x, q_w, k_w, v_w)
    res = run_bass_kernel_spmd(nc, in_maps, list(range(NCORES)), trace=trace)
    out = np.empty((B, SEQ, D), np.float32)
    for core in range(NCORES):
        b, h = divmod(core, 2)
        ot = res.results[core]["ot"].T.astype(np.float32)
        sm = res.results[core]["sm"].reshape(NQ).astype(np.float32)
        out[b, NQ * h : NQ * (h + 1)] = ot / sm[:, None]
    return out, res


def kernel(x, q_w, k_w, v_w):
    x = np.asarray(x, np.float32)
    q_w = np.asarray(q_w, np.float32)
    k_w = np.asarray(k_w, np.float32)
    v_w = np.asarray(v_w, np.float32)
    out, _ = run(x, q_w, k_w, v_w, trace=False)
    return out


# revision 24
# speedup vs baseline: 1.0377x; 1.0377x over previous
"""Single-head attention (B=4, N=2048, D=1024), scores scaled by 10.

Sharding: 8 cores = (batch, query-half). Core 2b+h owns queries
[1024h:1024(h+1)] of batch b. Pure SPMD — no collectives.

Algebra: scores = Q K^T = x_q (Wq^T Wk) x_k^T, so G = q_w^T @ k_w is
precomputed on host and the kernel computes U = x_q G on device; the
key side of QK^T is raw x. The V path is re-associated:
out = softmax(S) (x Wv) == (softmax(S) x) Wv, so each core computes
Y = P x_full then O = Y Wv locally — no V projection, no AllGathers,
no cross-core dependencies. Flop count is identical.

Key order is permuted (own query half first) so x_q is a column slice
of the x^T tile — softmax is permutation-invariant and Y uses the
same row permutation of x, so the output is unchanged.

Inputs are packed on host so every DMA is 128 partition-contiguous
multi-KB runs (desc-dominated 1KB loads made the old startup 14.5us),
split across both HWDGE rings (sync: x^T + x, scalar: G + Wv) so the
first matmul's operands land before the ~7.4us engine preamble ends.

Numerics: fp16 operands, fp32 PSUM (the x10 score scale needs fp16's
10-bit mantissa; measured rel err ~4.6e-3 vs the 2e-2 gate). The
1/sum normalization happens on the HOST: the kernel emits
unnormalized O^T plus per-query sum rows.

Schedule: QK runs in four 256-query chunks (N=256 matmuls measure
~111ns, at parity with N=512 per-flop; the fp32 score tiles then fit
SBUF with bufs=2). Scores stay k-partitioned so Y consumes P with no
transposes. Y processes chunk PAIRS: the two 256-halves accumulate
into one [128,512] PSUM bank with kt-interleaved matmuls (shares each
LDWEIGHTS across both halves and halves the PSUM->SBUF casts); O runs
N=512 per pair. Tensor stream: U0 U1 QK0 QK1 QK2 QK3 Y01 O01 Y23 O23
with each chunk's softmax (fold-max, rank-1 max broadcast, scale+exp)
interleaved into the following tensor block. Per-query sums ride the
otherwise-idle gpsimd engine. Queue discipline: input loads + fold
DMAs on the sync ring, G/Wv loads + output stores on the scalar ring,
sum rows on gpsimd.
"""

import numpy as np

B, SEQ, D = 4, 2048, 1024
NQ = 1024          # queries per core
QCH = 256          # attention q-chunk (QK granularity)
NCH = NQ // QCH    # 4
NCORES = 8
DT = D // 128      # 8 d-tiles
KT = SEQ // 128    # 16 k-tiles

_BUILT = {}


def _build():
    if "nc" in _BUILT:
        return _BUILT["nc"]
    from contextlib import ExitStack

    import concourse.bass as bass  # noqa: F401
    import concourse.mybir as mybir
    import concourse.tile as tile
    from concourse import bacc, bass_isa

    dt = mybir.dt
    F32, F16 = dt.float32, dt.float16
    AL = mybir.AluOpType
    EXP = mybir.ActivationFunctionType.Exp

    nc = bacc.Bacc("TRN2", target_bir_lowering=False, debug=False)

    # packed DRAM inputs: [128 partitions, contiguous per-partition payload]
    xk_d = nc.dram_tensor("xk", [128, 32 * 512], F16, kind="ExternalInput")
    xv_d = nc.dram_tensor("xv", [128, 128 * 128], F16, kind="ExternalInput")
    g_d = nc.dram_tensor("g", [128, 64 * 128], F16, kind="ExternalInput")
    wv_d = nc.dram_tensor("wv", [128, 64 * 128], F16, kind="ExternalInput")
    ot_d = nc.dram_tensor("ot", [D, NQ], F16, kind="ExternalOutput")
    sm_d = nc.dram_tensor("sm", [NCH * 4, QCH], F32, kind="ExternalOutput")

    xk_a = xk_d.ap().rearrange("p (t n) -> p t n", n=512)    # t = 8*chunk+dti
    xv_a = xv_d.ap().rearrange("p (t e) -> p t e", e=128)    # t = 8*kt+dti
    g_a = g_d.ap().rearrange("p (t e) -> p t e", e=128)      # t = 8*et+dti
    wv_a = wv_d.ap().rearrange("p (t e) -> p t e", e=128)
    ot_r = ot_d.ap().rearrange("(t p) q -> p t q", p=128)

    with tile.TileContext(nc) as tc, ExitStack() as ctx:
        main_pool = ctx.enter_context(tc.tile_pool(name="main", bufs=1))
        xk_t = main_pool.tile([128, 32, 512], F16, tag="xk")
        xv_t = main_pool.tile([128, 128, 128], F16, tag="xv")
        wv_t = main_pool.tile([128, 64, 128], F16, tag="wv")
        uth = main_pool.tile([128, DT, NQ], F16, tag="uth")
        p_t = main_pool.tile([128, KT, NQ], F16, tag="p")

        const_pool = ctx.enter_context(tc.tile_pool(name="const", bufs=1))
        ones_t = const_pool.tile([128, 1], F16, tag="ones")
        nc.vector.memset(ones_t[:], 1.0)

        # ---- Phase A: loads + U projection -------------------------------
        with (
            tc.tile_pool(name="gp", bufs=1) as gpool,
            tc.tile_pool(name="psA", bufs=4, space="PSUM") as psA,
            tc.tile_pool(name="psW", bufs=1, space="PSUM") as psW,
        ):
            g_t = gpool.tile([128, 64, 128], F16, tag="g")
            # ~3.6us of rank-1 warmups fill the initial DMA wait (PE would
            # be idle anyway) and flip the HAM clock gate to 2.4GHz before
            # the first real matmul
            warm = gpool.tile([128, 128], F16, tag="warm")
            nc.vector.memset(warm[:], 0.0)
            warm_ps = psW.tile([128, 128], F32, tag="warmps")
            for _ in range(44):
                nc.tensor.matmul(warm_ps[:], warm[:], warm[:], start=True, stop=True)
            # input loads, first-use order; two HWDGE rings in parallel.
            # The first U group's operands are split fine so its first
            # matmuls can start on half the data
            nc.sync.dma_start(xk_t[:, 0:4, :], xk_a[:, 0:4, :])      # xq c0 lo
            nc.sync.dma_start(xk_t[:, 4:8, :], xk_a[:, 4:8, :])      # xq c0 hi
            for et in range(DT):
                nc.scalar.dma_start(
                    g_t[:, 8 * et : 8 * et + 8, :], g_a[:, 8 * et : 8 * et + 8, :]
                )
            nc.sync.dma_start(xk_t[:, 8:16, :], xk_a[:, 8:16, :])    # xq chunk 1
            nc.sync.dma_start(xk_t[:, 16:24, :], xk_a[:, 16:24, :])  # other half
            nc.sync.dma_start(xk_t[:, 24:32, :], xk_a[:, 24:32, :])
            for j in range(4):
                nc.sync.dma_start(
                    xv_t[:, 32 * j : 32 * j + 32, :], xv_a[:, 32 * j : 32 * j + 32, :]
                )
            nc.scalar.dma_start(wv_t[:, 0:32, :], wv_a[:, 0:32, :])
            nc.scalar.dma_start(wv_t[:, 32:64, :], wv_a[:, 32:64, :])

            # U^T = (x_q G)^T, two 512-query chunks
            for chn in range(2):
                for et in range(DT):
                    ps = psA.tile([128, 512], F32, tag="psA")
                    for dti in range(DT):
                        nc.tensor.matmul(
                            ps[:],
                            g_t[:, 8 * et + dti, :],
                            xk_t[:, 8 * chn + dti, :],
                            start=(dti == 0),
                            stop=(dti == DT - 1),
                        )
                    nc.vector.tensor_copy(uth[:, et, 512 * chn : 512 * chn + 512], ps[:])

        # ---- Phase B: attention ------------------------------------------
        with (
            tc.tile_pool(name="stp", bufs=3) as stpool,
            tc.tile_pool(name="yp", bufs=1) as ypool,
            tc.tile_pool(name="aux", bufs=2) as auxpool,
            tc.tile_pool(name="osb", bufs=3) as outpool,
            tc.tile_pool(name="psS", bufs=3, space="PSUM") as psS,
            tc.tile_pool(name="psY", bufs=2, space="PSUM") as psY,
            tc.tile_pool(name="psO", bufs=2, space="PSUM") as psO,
            tc.tile_pool(name="psX", bufs=1, space="PSUM") as psX,
        ):
            def qk_group(c, st, kt, rmax):
                # QK^T for one k-tile; running per-query max rides the copy
                q0 = QCH * c
                ps = psS.tile([128, QCH], F32, tag="psS")
                for dti in range(DT):
                    nc.tensor.matmul(
                        ps[:],
                        xk_t[:, 8 * (kt // 4) + dti, 128 * (kt % 4) : 128 * (kt % 4) + 128],
                        uth[:, dti, q0 : q0 + QCH],
                        start=(dti == 0),
                        stop=(dti == DT - 1),
                    )
                nc.scalar.copy(st[:, kt, :], ps[:])
                if kt == 1:
                    nc.vector.tensor_max(rmax[:], st[:, 0, :], st[:, 1, :])
                elif kt >= 2:
                    nc.vector.tensor_max(rmax[:], rmax[:], st[:, kt, :])

            def reduce_bcast(c):
                # per-query max across the 128 key-partitions, broadcast to
                # all partitions, in ONE gpsimd op (replaces a ~6us serial
                # fold tree + rank-1 broadcast matmul on the PE FIFO)
                maxb_ = auxpool.tile([128, QCH], F32, tag="maxb", name=f"maxb{c}")
                nc.gpsimd.partition_all_reduce(
                    maxb_[:], rmax[c][:], 128, bass_isa.ReduceOp.max
                )
                return maxb_

            def stt_exp(c, st, maxb, j):
                # exp(10*s - 10*max) for k-tiles 4j..4j+3 into the shared P
                # buffer
                q0 = QCH * c
                for kt in range(4 * j, 4 * j + 4):
                    nc.vector.scalar_tensor_tensor(
                        st[:, kt, :],
                        st[:, kt, :],
                        1.0,
                        maxb[:],
                        op0=AL.mult,
                        op1=AL.subtract,
                    )
                for hh in range(2):
                    nc.scalar.activation(
                        p_t[:, 4 * j + 2 * hh : 4 * j + 2 * hh + 2, q0 : q0 + QCH],
                        st[:, 4 * j + 2 * hh : 4 * j + 2 * hh + 2, :],
                        EXP,
                    )

            def y_group(cp, y_t, dti):
                # Y^T d-tile for chunk PAIR cp: both 256-halves accumulate in
                # one [128,512] PSUM bank, kt-interleaved so each xv weight
                # load serves two matmuls; one cast out.
                q0 = 512 * cp
                ps = psY.tile([128, 512], F32, tag="psY")
                for kt in range(KT):
                    for h in range(2):
                        # start=True clears has_written BANK-wide, so only the
                        # very first matmul into the bank may set it (the h=1
                        # kt=0 half then overwrites its still-clear elements)
                        nc.tensor.matmul(
                            ps[:, 256 * h : 256 * h + 256],
                            xv_t[:, 8 * kt + dti, :],
                            p_t[:, kt, q0 + 256 * h : q0 + 256 * h + 256],
                            start=(kt == 0 and h == 0),
                            stop=(kt == KT - 1 and h == 1),
                        )
                nc.vector.tensor_copy(y_t[:, dti, :], ps[:])

            def o_group(cp, y_t, et):
                # O^T e-tile for chunk pair cp at N=512; store via scalar ring
                q0 = 512 * cp
                ps = psO.tile([128, 512], F32, tag="psO")
                for dti in range(DT):
                    nc.tensor.matmul(
                        ps[:],
                        wv_t[:, 8 * et + dti, :],
                        y_t[:, dti, :],
                        start=(dti == 0),
                        stop=(dti == DT - 1),
                    )
                osb = outpool.tile([128, 512], F16, tag="osb")
                nc.vector.tensor_copy(osb[:], ps[:])
                nc.scalar.dma_start(ot_r[:, et, q0 : q0 + 512], osb[:])

            def sum_mms(c):
                # per-query sums as rank-1 matmuls (ones^T @ P). Four 4-kt
                # partial chains run CONCURRENTLY in distinct 32-col groups
                # of the PE array (tile_position col-tiling), each into its
                # own partition row of one PSUM bank; host adds the 4 rows.
                # start/stop per chain: the has_written clear covers the
                # targeted partition group (x full free range), so disjoint
                # partition chains each manage their own group.
                q0 = QCH * c
                qs = psX.tile([128, QCH], F32, tag="qsum")
                for a in range(4):
                    for kt in range(4 * a, 4 * a + 4):
                        nc.tensor.matmul(
                            qs[32 * a : 32 * a + 1, :],
                            ones_t[:],
                            p_t[:, kt, q0 : q0 + QCH],
                            start=(kt == 4 * a),
                            stop=(kt == 4 * a + 3),
                            tile_position=(0, 32 * a),
                        )
                srow = auxpool.tile([128, QCH], F32, tag="srow")
                nc.vector.tensor_copy(srow[:], qs[:])
                srows = srow[:].rearrange("(a b) q -> a b q", b=32)
                nc.gpsimd.dma_start(sm_d.ap()[4 * c : 4 * c + 4, :], srows[:, 0, :])

            st = [None] * NCH
            y = [None] * 2
            rmax = [None] * NCH
            maxb = [None] * NCH

            def new_chunk(c):
                st[c] = stpool.tile([128, KT, QCH], F32, tag="st", name=f"st{c}")
                rmax[c] = auxpool.tile([128, QCH], F32, tag="rmax", name=f"rmax{c}")

            def qk_run(c, kts):
                for kt in kts:
                    qk_group(c, st[c], kt, rmax[c])

            def exp_batch(c, j):
                stt_exp(c, st[c], maxb[c], j)

            def y_run(cp, dtis):
                if y[cp] is None:
                    y[cp] = ypool.tile([128, DT, 512], F16, tag="y", name=f"y{cp}")
                for dti in dtis:
                    y_group(cp, y[cp], dti)

            # ---- interleaved schedule ----
            # tensor: QK0 QK1 QK2 QK3 Y01 O01 Y23 O23. Each chunk's softmax
            # is spread over the following TWO half-blocks so the vector
            # engine (scale+sub, running-max, fold trees) never outruns its
            # window; st bufs=3 keeps the pool rotation ahead of the spread.
            new_chunk(0)
            qk_run(0, range(KT))
            maxb[0] = reduce_bcast(0)
            new_chunk(1)
            qk_run(1, range(0, 7))
            exp_batch(0, 0)
            qk_run(1, range(7, 11))
            exp_batch(0, 1)
            qk_run(1, range(11, KT))
            maxb[1] = reduce_bcast(1)
            new_chunk(2)
            qk_run(2, range(0, 3))
            exp_batch(0, 2)
            qk_run(2, range(3, 6))
            exp_batch(0, 3)
            qk_run(2, range(6, 9))
            exp_batch(1, 0)
            qk_run(2, range(9, 12))
            exp_batch(1, 1)
            qk_run(2, range(12, KT))
            maxb[2] = reduce_bcast(2)
            new_chunk(3)
            qk_run(3, range(0, 3))
            exp_batch(1, 2)
            qk_run(3, range(3, 5))
            sum_mms(0)
            exp_batch(1, 3)
            qk_run(3, range(5, 8))
            exp_batch(2, 0)
            qk_run(3, range(8, 11))
            exp_batch(2, 1)
            qk_run(3, range(11, KT))
            maxb[3] = reduce_bcast(3)
            y_run(0, range(0, 1))
            exp_batch(2, 2)
            y_run(0, range(1, 2))
            sum_mms(1)
            exp_batch(2, 3)
            y_run(0, range(2, 3))
            exp_batch(3, 0)
            y_run(0, range(3, 5))
            exp_batch(3, 1)
            y_run(0, range(5, DT))
            o_group(0, y[0], 0)
            exp_batch(3, 2)
            o_group(0, y[0], 1)
            sum_mms(2)
            exp_batch(3, 3)
            for et in range(2, DT):
                o_group(0, y[0], et)
            y_run(1, range(0, 1))
            sum_mms(3)
            y_run(1, range(1, DT))
            for et in range(DT - 1):
                o_group(1, y[1], et)
            ps_l = psO.tile([128, 512], F32, tag="psO", name="ps_last")
            for h in range(2):
                for dti in range(DT):
                    nc.tensor.matmul(
                        ps_l[:, 256 * h : 256 * h + 256],
                        wv_t[:, 8 * (DT - 1) + dti, :],
                        y[1][:, dti, 256 * h : 256 * h + 256],
                        start=(dti == 0 and h == 0),
                        stop=(dti == DT - 1 and h == 1),
                    )
                osb_h = outpool.tile([128, 256], F16, tag="osbh", name=f"osbh{h}")
                nc.vector.tensor_copy(osb_h[:], ps_l[:, 256 * h : 256 * h + 256])
                nc.scalar.dma_start(
                    ot_r[:, DT - 1, 512 + 256 * h : 512 + 256 * h + 256], osb_h[:]
                )

    nc.compile()
    _BUILT["nc"] = nc
    return nc


def _prep_inputs(x, q_w, k_w, v_w):
    f16 = np.float16
    G = (10.0 * (q_w.T @ k_w)).astype(f16)
    g_pack = np.ascontiguousarray(
        G.reshape(8, 128, 8, 128).transpose(1, 2, 0, 3).reshape(128, 64 * 128)
    )
    wv = v_w.T.astype(f16)
    wv_pack = np.ascontiguousarray(
        wv.reshape(8, 128, 8, 128).transpose(1, 2, 0, 3).reshape(128, 64 * 128)
    )

    in_maps = []
    for core in range(NCORES):
        b, h = divmod(core, 2)
        xb = np.asarray(x[b]).astype(f16)                    # [2048, 1024]
        xp = np.concatenate([xb[NQ * h : NQ * (h + 1)], xb[NQ * (1 - h) : NQ * (2 - h)]])
        xk_pack = np.ascontiguousarray(
            xp.T.reshape(8, 128, 4, 512).transpose(1, 2, 0, 3).reshape(128, 32 * 512)
        )
        xv_pack = np.ascontiguousarray(
            xp.reshape(16, 128, 8, 128).transpose(1, 0, 2, 3).reshape(128, 128 * 128)
        )
        in_maps.append({"xk": xk_pack, "xv": xv_pack, "g": g_pack, "wv": wv_pack})
    return in_maps


def run(x, q_w, k_w, v_w, trace=False):
    from concourse.bass_utils import run_bass_kernel_spmd

    nc = _build()
    in_maps = _prep_inputs(x, q_w, k_w, v_w)
    res = run_bass_kernel_spmd(nc, in_maps, list(range(NCORES)), trace=trace)
    out = np.empty((B, SEQ, D), np.float32)
    for core in range(NCORES):
        b, h = divmod(core, 2)
        ot = res.results[core]["ot"].T.astype(np.float32)
        sm = (
            res.results[core]["sm"].reshape(NCH, 4, QCH).sum(1).reshape(NQ)
        ).astype(np.float32)
        out[b, NQ * h : NQ * (h + 1)] = ot / sm[:, None]
    return out, res


def kernel(x, q_w, k_w, v_w):
    x = np.asarray(x, np.float32)
    q_w = np.asarray(q_w, np.float32)
    k_w = np.asarray(k_w, np.float32)
    v_w = np.asarray(v_w, np.float32)
    out, _ = run(x, q_w, k_w, v_w, trace=False)
    return out
